# revision 31
# baseline (speedup 1.0000x reference)
"""Trainium2 Bass kernel v5 — paired K=64 attention + ScalarE/DVE exp split.

Multi-head attention (B=2, N=4096, D=768, H=12, d_head=64) on 8 NeuronCores.
Data-parallel over batch (4 cores per element), tensor-parallel over heads
(3 heads per core). Host sums the 4 partial outputs per batch element.

Key design points (each bench-validated on HW):
  * Softmax drain split: every [128, 1024] score tile leaves PSUM through
    exactly one instruction — ScalarE true exp (even key-chunks, 1113ns)
    or DVE Schraudolph fast-exp (odd chunks, one tensor_scalar:
    p = bitcast_fp16(int16(s * SCALE*1024/ln2 + 15360-61.5)), 1223ns).
    The -61.5 zeroes the MEAN multiplicative bias of the +-4% sawtooth —
    a systematic bias between DVE and ACT chunks would NOT cancel in the
    softmax normalization. Final rel-err 9.3e-3 vs the 2e-2 gate.
  * Score matmuls pair two K=64 heads in the 128-row PE array via
    tile_position (0,0)/(64,0) row tiling (concurrent, ~4ns apart): the
    h0/h1 passes pair the heads; the h2 pass pairs two query halves
    against duplicated k2/q2 partition halves.
  * sps bufs=3 sets the s-tile ring depth that hides the ~1.7us
    sem+exp latency chain; ops bufs=1 works because ScalarE frees o
    (rows 0..64 -> SBUF) immediately after the AV accumulation stops.
  * Inputs are cast to fp16 on the host and DMA straight into fp16 SBUF
    tiles (no on-device cast stream); k2 reaches its kT2z partition rows
    via an SBUF->SBUF DMA (engines cannot cross partitions, DMA can).
  * Normalize chain: ScalarE o-copy, DVE reciprocal (input MUST sit at
    base partition 0 — feeding a base-64 AP produced NaNs on HW),
    GpSimd broadcast + multiply into A_z.
  * Output projection token-chunks trickle into the PE stream mid-pass
    (kc 10/20/30) to fill exp-chain stalls.

Layouts (fp16): qT01/kT01 [128, N]: h0 rows 0..63, h1 rows 64..127.
qT2/kT2z [128, N]: q2/k2 duplicated in both 64-row halves. V
[128, NKC, 3, 128]: per (kc, h): cols 0..63 = V, col 64 = ones (softmax
denominator via the AV matmul), rest zeros. A_z[h] [128, N]: normalized
attention rows 0..63, h2 row 64 = ones (bias row). wout_z[h] [128, D]:
W_out rows for head h in 0..63 (+bias row 64 for h2).
wqkv host layout [768, 576]: [q01(128) | k01(128) | q2k2(128) | v(192)].
"""

import numpy as np

import concourse.bass as bass
import concourse.tile as tile
from concourse import mybir, bacc
from concourse.bass_utils import run_bass_kernel_spmd

F32 = mybir.dt.float32
F16 = mybir.dt.float16
I16 = mybir.dt.int16
EXP = mybir.ActivationFunctionType.Exp
MULT = mybir.AluOpType.mult
ADD = mybir.AluOpType.add

N_CORES = 8
B = 2
N = 4096
D = 768
H = 12
HD = 64  # head dim
SCALE = HD ** -0.5
DC = D // 128       # 6 contraction chunks
QC = 1024           # query block
NQC = N // QC       # 4
NKC = N // 128      # 32 key chunks

# Schraudolph fp16 exp: p = bitcast_fp16(int16(x*EA + EB)); EA folds the
# softmax 1/sqrt(d) scale. EB zeroes the MEAN multiplicative bias (a
# systematic bias between DVE- and ACT-computed key chunks would not
# cancel in the softmax normalization).
EXP_A = SCALE * 1024.0 / np.log(2.0)
EXP_B = 15 * 1024.0 - 61.5
# Which key-chunks the DVE takes (ScalarE true-exp handles the rest).
# Even split: ScalarE's faster per-tile drain (1336ns vs 1469ns) offsets
# the o->SBUF normalize copies and y drains it also carries.
DVE_KC = set(range(1, NKC, 2))

TRACE = False
TRACE_ALL_CORES = False
LAST_RESULT = None

_nc_cache = None


def _build_module():
    nc = bacc.Bacc("TRN2", target_bir_lowering=False, debug=False,
                   num_devices=N_CORES)
    x_d = nc.dram_tensor("x", [D, N], F16, kind="ExternalInput")
    wqkv_d = nc.dram_tensor("wqkv", [D, 576], F16, kind="ExternalInput")
    wout_d = nc.dram_tensor("wout", [193, D], F16, kind="ExternalInput")
    y_d = nc.dram_tensor("y", [N, D], F32, kind="ExternalOutput")

    with tile.TileContext(nc) as tc:
        _emit(nc, tc, x_d, wqkv_d, wout_d, y_d)
    nc.compile()
    return nc


def _emit(nc, tc, x_d, wqkv_d, wout_d, y_d):
    from contextlib import ExitStack
    ctx = ExitStack()
    with ctx:
        weights = ctx.enter_context(tc.tile_pool(name="weights", bufs=1))
        qkvp = ctx.enter_context(tc.tile_pool(name="qkv", bufs=1))
        apool = ctx.enter_context(tc.tile_pool(name="attnout", bufs=1))

        # --- weights / persistent activations ---------------------------
        wqkv = weights.tile([128, DC, 576], F16, tag="wqkv")
        wout01 = weights.tile([128, D], F16, tag="wout01")
        wout2z = weights.tile([65, D], F16, tag="wout2z")
        qT01 = qkvp.tile([128, N], F16, tag="qT01")
        qT2 = qkvp.tile([128, N], F16, tag="qT2")
        kT01 = qkvp.tile([128, N], F16, tag="kT01")
        kT2z = qkvp.tile([128, N], F16, tag="kT2z")
        V = qkvp.tile([128, NKC, 3, 128], F16, tag="V")
        A01 = apool.tile([128, N], F16, tag="A01")
        A2z = apool.tile([65, N], F16, tag="A2z")

        # --- weight DMAs (straight into fp16 tiles) ----------------------
        nc.sync.dma_start(
            wqkv[:], wqkv_d.ap().rearrange("(c p) m -> p c m", p=128))
        nc.sync.dma_start(wout01[:], wout_d.ap()[0:128, :])
        nc.sync.dma_start(wout2z[:], wout_d.ap()[128:193, :])

        # zero-fill padded halves (one-time, GpSimd) + ones columns/rows
        nc.gpsimd.memset(V[:], 0.0)
        nc.vector.memset(V[:, :, 0, 64:65], 1.0)
        nc.vector.memset(V[:, :, 1, 0:1], 1.0)
        nc.vector.memset(V[:, :, 2, 64:65], 1.0)
        nc.vector.memset(A2z[64:65, :], 1.0)

        # ================= phase A: fused qkv projections ================
        with tc.tile_pool(name="xT", bufs=2) as xTp, \
             tc.tile_pool(name="ktmp", bufs=2) as ktp, \
             tc.tile_pool(name="vps", bufs=2, space=bass.MemorySpace.PSUM) as vps, \
             tc.tile_pool(name="qkps", bufs=2, space=bass.MemorySpace.PSUM) as qkps:
            NSEG = 4
            SEG = N // NSEG
            SEGC = SEG // 128
            for seg in range(NSEG):
                t0 = seg * SEGC
                col0 = seg * SEG
                xT = xTp.tile([128, DC, SEG], F16, tag="xT")
                nc.sync.dma_start(
                    xT[:],
                    x_d.ap().rearrange("(c p) n -> p c n", p=128)
                    [:, :, col0:col0 + SEG])
                # fused projections; (k and v first so attention can start
                # before q finishes). copies: (dst, rows, engine).
                # ci=2 is the shared q2|k2 block: q2 sits in psum rows 0..63
                # (straight copy); k2 lands in rows 64..127 and is shifted to
                # kT2z rows 0..63 by an SBUF->SBUF DMA (engines cannot move
                # data across partitions; DMA can).
                for ci, copies in (
                        (1, ((kT01, 0, 128, nc.scalar),)),
                        (2, None),
                        (-1, ()),
                        (0, ((qT01, 0, 128, nc.scalar),))):
                    if ci == -1:
                        for g in range(SEGC // 4):
                            acc = vps.tile([128, 4, 4, 64], F32, tag="vps")
                            for t4 in range(4):
                                t = g * 4 + t4
                                for c in range(DC):
                                    nc.tensor.matmul(
                                        acc[:, t4, 0:3, :],
                                        xT[:, c, t * 128:(t + 1) * 128],
                                        wqkv[:, c, 384:576],
                                        start=(c == 0), stop=(c == DC - 1))
                            tg = t0 + g * 4
                            nc.scalar.copy(V[:, tg:tg + 4, 0:3:2, 0:64],
                                           acc[:, :, 0:3:2, :])
                            nc.scalar.copy(V[:, tg:tg + 4, 1, 64:128],
                                           acc[:, :, 1, :])
                        continue
                    c0 = 128 * ci
                    acc = qkps.tile([128, SEG], F32, tag="qkps")
                    # c outer: the stationary wqkv chunk is loaded once per c
                    # and reused for both nb half-banks
                    for c in range(DC):
                        for nb in range(SEG // 512):
                            nc.tensor.matmul(
                                acc[:, nb * 512:(nb + 1) * 512],
                                wqkv[:, c, c0:c0 + 128],
                                xT[:, c, nb * 512:(nb + 1) * 512],
                                start=(c == 0), stop=(c == DC - 1))
                    if copies is None:
                        # q2/k2 are DUPLICATED into both partition halves so
                        # the h2 pass can run self-paired row-tiled matmuls
                        # (two query halves concurrently on the K=64 head).
                        nc.vector.tensor_copy(qT2[0:64, col0:col0 + SEG],
                                              acc[0:64, :])
                        nc.sync.dma_start(qT2[64:128, col0:col0 + SEG],
                                          qT2[0:64, col0:col0 + SEG])
                        ktmp = ktp.tile([128, SEG], F16, tag="ktmp")
                        nc.scalar.copy(ktmp[64:128, :], acc[64:128, :])
                        nc.vector.tensor_copy(kT2z[64:128, col0:col0 + SEG],
                                              ktmp[64:128, :])
                        nc.sync.dma_start(kT2z[0:64, col0:col0 + SEG],
                                          ktmp[64:128, :])
                        continue
                    for dst, lo, hi, eng in copies:
                        if eng is nc.scalar:
                            nc.scalar.copy(dst[lo:hi, col0:col0 + SEG],
                                           acc[lo:hi, :])
                        else:
                            nc.vector.tensor_copy(dst[lo:hi, col0:col0 + SEG],
                                                  acc[lo:hi, :])

        # ========= phase B: flash attention + fused output projection ====
        with tc.tile_pool(name="sps", bufs=3, space=bass.MemorySpace.PSUM) as sps, \
             tc.tile_pool(name="ops", bufs=1, space=bass.MemorySpace.PSUM) as ops, \
             tc.tile_pool(name="pp", bufs=4) as pp, \
             tc.tile_pool(name="ysbp", bufs=3) as ysbp, \
             tc.tile_pool(name="osbp", bufs=2) as osbp, \
             tc.tile_pool(name="rp", bufs=2) as rp, \
             tc.tile_pool(name="rbp", bufs=2) as rbp:
            pending = []   # deferred output-projection token chunks

            def emit_y(t):
                ts = slice(t * 128, (t + 1) * 128)
                y = sps.tile([128, D], F32, tag="s", name="y")
                # A01 carries h0 (rows 0..63) and h1 (rows 64..127) so one
                # K=128 contraction sums both heads; A2z is K=65 (h2+bias)
                for hh, (az, w) in enumerate(((A01, wout01), (A2z, wout2z))):
                    for c0, c1 in ((0, 512), (512, 768)):
                        nc.tensor.matmul(y[:, c0:c1], az[:, ts],
                                         w[:, c0:c1],
                                         start=(hh == 0), stop=(hh == 1))
                ysb = ysbp.tile([128, D], F32, tag="ysb", name="ysb")
                nc.scalar.copy(ysb[:], y[:])
                nc.sync.dma_start(y_d.ap()[ts, :], ysb[:])

            def exp_drain(p, s, kc):
                if kc in DVE_KC:
                    nc.vector.tensor_scalar(
                        p[:].bitcast(I16), s[:], EXP_A, EXP_B, MULT, ADD)
                else:
                    nc.scalar.activation(p[:], s[:], EXP, scale=SCALE)

            def normalize_pair(o2, q0):
                # o2 layout: cols 0:512 = h0 (rows 0..63 o, row 64 den),
                # cols 512:1024 = h1 (row 0 den, rows 64..127 o). ScalarE
                # frees the bank; DVE gathers dens at base partition 0
                # (recip input must sit at base 0 — base-64 APs NaN on HW);
                # GpSimd broadcasts + multiplies into A01's row halves.
                osb = osbp.tile([128, 2 * QCP], F32, tag="osb", name="osb")
                nc.scalar.copy(osb[:], o2[:, :])
                den = rp.tile([1, 2 * QCP], F32, tag="den")
                nc.vector.tensor_copy(den[:, 0:QCP], osb[64:65, 0:QCP])
                nc.vector.tensor_copy(den[:, QCP:], osb[0:1, QCP:])
                rc = rp.tile([1, 2 * QCP], F32, tag="rc")
                nc.vector.reciprocal_approx_fast(rc[:], den[:])
                rcb = rbp.tile([128, 2 * QCP], F32, tag="rcb")
                nc.gpsimd.partition_broadcast(rcb[:], rc[:])
                nc.gpsimd.tensor_mul(A01[0:64, q0:q0 + QCP],
                                     osb[0:64, 0:QCP], rcb[0:64, 0:QCP])
                nc.gpsimd.tensor_mul(A01[64:128, q0:q0 + QCP],
                                     osb[64:128, QCP:], rcb[64:128, QCP:])

            def normalize_h2(o, q0):
                osb = osbp.tile([65, QC], F32, tag="osb2", name="osb2")
                nc.scalar.copy(osb[:], o[0:65, :])
                den = rp.tile([1, QC], F32, tag="den")
                nc.vector.tensor_copy(den[:], osb[64:65, :])
                rc = rp.tile([1, QC], F32, tag="rc")
                nc.vector.reciprocal_approx_fast(rc[:], den[:])
                rcb = rbp.tile([64, QC], F32, tag="rcb")
                nc.gpsimd.partition_broadcast(rcb[:], rc[:])
                nc.gpsimd.tensor_mul(A2z[0:64, q0:q0 + QC],
                                     osb[0:64, :], rcb[:])

            QCP = 512  # pair-pass query block (h0+h1 share a 1024-wide tile)

            def pair_pass(qbb, half):
                q0 = qbb * QC + half * QCP
                o2 = ops.tile([128, 2 * QCP], F32, tag="o")
                for kc in range(NKC):
                    kk = slice(kc * 128, (kc + 1) * 128)
                    s2 = sps.tile([128, 2 * QCP], F32, tag="s")
                    nc.tensor.matmul(s2[:, 0:QCP], kT01[0:64, kk],
                                     qT01[0:64, q0:q0 + QCP],
                                     start=True, stop=True,
                                     tile_position=(0, 0))
                    nc.tensor.matmul(s2[:, QCP:], kT01[64:128, kk],
                                     qT01[64:128, q0:q0 + QCP],
                                     start=True, stop=True,
                                     tile_position=(64, 0))
                    p = pp.tile([128, 2 * QCP], F16, tag="p")
                    exp_drain(p, s2, kc)
                    nc.tensor.matmul(o2[:, 0:QCP], V[:, kc, 0, :],
                                     p[:, 0:QCP],
                                     start=(kc == 0), stop=(kc == NKC - 1))
                    nc.tensor.matmul(o2[:, QCP:], V[:, kc, 1, :],
                                     p[:, QCP:],
                                     start=(kc == 0), stop=(kc == NKC - 1))
                    if pending and kc in (10, 20, 30):
                        emit_y(pending.pop(0))
                normalize_pair(o2, q0)

            def h2_pass(qbb):
                q0 = qbb * QC
                o = ops.tile([128, QC], F32, tag="o")
                for kc in range(NKC):
                    kk = slice(kc * 128, (kc + 1) * 128)
                    s = sps.tile([128, QC], F32, tag="s")
                    # self-paired: both query halves of h2 run concurrently
                    # in the two 64-row array halves (k2/q2 duplicated there)
                    nc.tensor.matmul(s[:, 0:512], kT2z[0:64, kk],
                                     qT2[0:64, q0:q0 + 512],
                                     start=True, stop=True,
                                     tile_position=(0, 0))
                    nc.tensor.matmul(s[:, 512:1024], kT2z[64:128, kk],
                                     qT2[64:128, q0 + 512:q0 + 1024],
                                     start=True, stop=True,
                                     tile_position=(64, 0))
                    p = pp.tile([128, QC], F16, tag="p")
                    exp_drain(p, s, kc)
                    for j in (0, 512):
                        nc.tensor.matmul(o[:, j:j + 512], V[:, kc, 2, :],
                                         p[:, j:j + 512],
                                         start=(kc == 0), stop=(kc == NKC - 1))
                    if pending and kc in (10, 20, 30):
                        emit_y(pending.pop(0))
                normalize_h2(o, q0)

            for qbb in range(NQC):
                if qbb < NQC - 1:
                    pair_pass(qbb, 0)
                    pair_pass(qbb, 1)
                    h2_pass(qbb)
                    pending.extend(range(qbb * 8, qbb * 8 + 8))
                else:
                    # last block: h2 first so its A_z is ready early, letting
                    # the first half's output chunks trickle into the second
                    # pair pass instead of serializing at the very end
                    h2_pass(qbb)
                    pair_pass(qbb, 0)
                    pending.extend(range(qbb * 8, qbb * 8 + 4))
                    pair_pass(qbb, 1)
                    pending.extend(range(qbb * 8 + 4, qbb * 8 + 8))
            for t in pending:
                emit_y(t)


def _get_nc():
    global _nc_cache
    if _nc_cache is None:
        _nc_cache = _build_module()
    return _nc_cache


def kernel(x, W_qkv, W_out, b_out):
    global LAST_RESULT
    x = np.asarray(x, dtype=np.float32)
    W_qkv = np.asarray(W_qkv, dtype=np.float32)
    W_out = np.asarray(W_out, dtype=np.float32)
    b_out = np.asarray(b_out, dtype=np.float32)

    in_maps = []
    for c in range(N_CORES):
        b, j = divmod(c, 4)
        h0 = 3 * j
        q0, k0, v0 = 64 * h0, D + 64 * h0, 2 * D + 64 * h0
        q01 = W_qkv[:, q0:q0 + 128]
        k01 = W_qkv[:, k0:k0 + 128]
        q2 = W_qkv[:, q0 + 128:q0 + 192]
        k2 = W_qkv[:, k0 + 128:k0 + 192]
        v012 = W_qkv[:, v0:v0 + 192]
        wqkv_slice = np.ascontiguousarray(np.concatenate(
            [q01, k01, q2, k2, v012], axis=1).astype(np.float16))
        r0 = 64 * h0
        bias_row = b_out[None, :] if j == 0 else np.zeros((1, D), np.float32)
        wout_slice = np.ascontiguousarray(np.concatenate(
            [W_out[r0:r0 + 192], bias_row], axis=0).astype(np.float16))
        in_maps.append({
            "x": np.ascontiguousarray(x[b].T.astype(np.float16)),
            "wqkv": wqkv_slice,
            "wout": wout_slice,
        })

    nc = _get_nc()
    kwargs = {}
    if TRACE:
        from concourse import bass_utils as _bu
        _bu.upload_artifacts = lambda tmpdir: "local://" + tmpdir
        kwargs["trace"] = True
        if TRACE_ALL_CORES:
            kwargs["trace_cores"] = list(range(N_CORES))
    res = run_bass_kernel_spmd(nc, in_maps, core_ids=list(range(N_CORES)), **kwargs)
    LAST_RESULT = res

    out = np.empty((B, N, D), dtype=np.float32)
    for b in range(B):
        out[b] = (res.results[4 * b + 0]["y"] + res.results[4 * b + 1]["y"]
                  + res.results[4 * b + 2]["y"] + res.results[4 * b + 3]["y"])
    return out


# revision 32
# speedup vs baseline: 1.0943x; 1.0943x over previous
"""Trainium2 Bass kernel v5 — paired K=64 attention + ScalarE/DVE exp split.

Multi-head attention (B=2, N=4096, D=768, H=12, d_head=64) on 8 NeuronCores.
Data-parallel over batch (4 cores per element), tensor-parallel over heads
(3 heads per core). Host sums the 4 partial outputs per batch element.

Key design points (each bench-validated on HW):
  * Softmax drain split: every [128, 1024] score tile leaves PSUM through
    exactly one instruction — ScalarE true exp (even key-chunks, 1113ns)
    or DVE Schraudolph fast-exp (odd chunks, one tensor_scalar:
    p = bitcast_fp16(int16(s * SCALE*1024/ln2 + 15360-61.5)), 1223ns).
    The -61.5 zeroes the MEAN multiplicative bias of the +-4% sawtooth —
    a systematic bias between DVE and ACT chunks would NOT cancel in the
    softmax normalization. Final rel-err 9.3e-3 vs the 2e-2 gate.
  * Score matmuls pair two K=64 heads in the 128-row PE array via
    tile_position (0,0)/(64,0) row tiling (concurrent, ~4ns apart): the
    h0/h1 passes pair the heads; the h2 pass pairs two query halves
    against duplicated k2/q2 partition halves.
  * sps bufs=3 sets the s-tile ring depth that hides the ~1.7us
    sem+exp latency chain; ops bufs=1 works because ScalarE frees o
    (rows 0..64 -> SBUF) immediately after the AV accumulation stops.
  * Inputs are cast to fp16 on the host and DMA straight into fp16 SBUF
    tiles (no on-device cast stream); k2 reaches its kT2z partition rows
    via an SBUF->SBUF DMA (engines cannot cross partitions, DMA can).
  * Normalize chain: ScalarE o-copy, DVE reciprocal (input MUST sit at
    base partition 0 — feeding a base-64 AP produced NaNs on HW),
    GpSimd broadcast + multiply into A_z.
  * Output projection token-chunks trickle into the PE stream mid-pass
    (kc 10/20/30) to fill exp-chain stalls.

Layouts (fp16): qT01/kT01 [128, N]: h0 rows 0..63, h1 rows 64..127.
qT2/kT2z [128, N]: q2/k2 duplicated in both 64-row halves. V
[128, NKC, 3, 128]: per (kc, h): cols 0..63 = V, col 64 = ones (softmax
denominator via the AV matmul), rest zeros. A_z[h] [128, N]: normalized
attention rows 0..63, h2 row 64 = ones (bias row). wout_z[h] [128, D]:
W_out rows for head h in 0..63 (+bias row 64 for h2).
wqkv host layout [768, 576]: [q01(128) | k01(128) | q2k2(128) | v(192)].
"""

import numpy as np

import concourse.bass as bass
import concourse.tile as tile
from concourse import mybir, bacc
from concourse.bass_utils import run_bass_kernel_spmd

F32 = mybir.dt.float32
F16 = mybir.dt.float16
I16 = mybir.dt.int16
EXP = mybir.ActivationFunctionType.Exp
MULT = mybir.AluOpType.mult
ADD = mybir.AluOpType.add

N_CORES = 8
B = 2
N = 4096
D = 768
H = 12
HD = 64  # head dim
SCALE = HD ** -0.5
DC = D // 128       # 6 contraction chunks
QC = 1024           # query block
NQC = N // QC       # 4
NKC = N // 128      # 32 key chunks

# Schraudolph fp16 exp: p = bitcast_fp16(int16(x*EA + EB)); EA folds the
# softmax 1/sqrt(d) scale. EB zeroes the MEAN multiplicative bias (a
# systematic bias between DVE- and ACT-computed key chunks would not
# cancel in the softmax normalization).
EXP_A = SCALE * 1024.0 / np.log(2.0)
EXP_B = 15 * 1024.0 - 61.5
# Which key-chunks the DVE takes (ScalarE true-exp handles the rest).
# Even split: ScalarE's faster per-tile drain (1336ns vs 1469ns) offsets
# the o->SBUF normalize copies and y drains it also carries.
DVE_KC = set(range(1, NKC, 2)) | {16}

TRACE = False
TRACE_ALL_CORES = False
LAST_RESULT = None

_nc_cache = None


def _build_module():
    nc = bacc.Bacc("TRN2", target_bir_lowering=False, debug=False,
                   num_devices=N_CORES)
    x_d = nc.dram_tensor("x", [D, N], F16, kind="ExternalInput")
    wqkv_d = nc.dram_tensor("wqkv", [D, 576], F16, kind="ExternalInput")
    wout_d = nc.dram_tensor("wout", [193, D], F16, kind="ExternalInput")
    y_d = nc.dram_tensor("y", [N, D], F32, kind="ExternalOutput")

    with tile.TileContext(nc) as tc:
        _emit(nc, tc, x_d, wqkv_d, wout_d, y_d)
    nc.compile()
    return nc


def _emit(nc, tc, x_d, wqkv_d, wout_d, y_d):
    from contextlib import ExitStack
    ctx = ExitStack()
    with ctx:
        weights = ctx.enter_context(tc.tile_pool(name="weights", bufs=1))
        qkvp = ctx.enter_context(tc.tile_pool(name="qkv", bufs=1))
        apool = ctx.enter_context(tc.tile_pool(name="attnout", bufs=1))

        # --- weights / persistent activations ---------------------------
        wqkv = weights.tile([128, DC, 576], F16, tag="wqkv")
        wout01 = weights.tile([128, D], F16, tag="wout01")
        wout2z = weights.tile([65, D], F16, tag="wout2z")
        qT01 = qkvp.tile([128, N], F16, tag="qT01")
        qT2 = qkvp.tile([128, N], F16, tag="qT2")
        kT01 = qkvp.tile([128, N], F16, tag="kT01")
        kT2z = qkvp.tile([128, N], F16, tag="kT2z")
        V = qkvp.tile([128, NKC, 3, 128], F16, tag="V")
        A01 = apool.tile([128, N], F16, tag="A01")
        A2z = apool.tile([65, N], F16, tag="A2z")

        # --- weight DMAs (straight into fp16 tiles) ----------------------
        nc.sync.dma_start(
            wqkv[:], wqkv_d.ap().rearrange("(c p) m -> p c m", p=128))
        nc.sync.dma_start(wout01[:], wout_d.ap()[0:128, :])
        nc.sync.dma_start(wout2z[:], wout_d.ap()[128:193, :])

        # zero-fill padded halves (one-time, GpSimd) + ones columns/rows
        nc.gpsimd.memset(V[:], 0.0)
        nc.vector.memset(V[:, :, 0, 64:65], 1.0)
        nc.vector.memset(V[:, :, 1, 0:1], 1.0)
        nc.vector.memset(V[:, :, 2, 64:65], 1.0)
        nc.vector.memset(A2z[64:65, :], 1.0)

        # ================= phase A: fused qkv projections ================
        with tc.tile_pool(name="xT", bufs=2) as xTp, \
             tc.tile_pool(name="ktmp", bufs=2) as ktp, \
             tc.tile_pool(name="vps", bufs=2, space=bass.MemorySpace.PSUM) as vps, \
             tc.tile_pool(name="qkps", bufs=2, space=bass.MemorySpace.PSUM) as qkps:
            NSEG = 4
            SEG = N // NSEG
            SEGC = SEG // 128
            for seg in range(NSEG):
                t0 = seg * SEGC
                col0 = seg * SEG
                xT = xTp.tile([128, DC, SEG], F16, tag="xT")
                nc.sync.dma_start(
                    xT[:],
                    x_d.ap().rearrange("(c p) n -> p c n", p=128)
                    [:, :, col0:col0 + SEG])
                # fused projections; (k and v first so attention can start
                # before q finishes). copies: (dst, rows, engine).
                # ci=2 is the shared q2|k2 block: q2 sits in psum rows 0..63
                # (straight copy); k2 lands in rows 64..127 and is shifted to
                # kT2z rows 0..63 by an SBUF->SBUF DMA (engines cannot move
                # data across partitions; DMA can).
                for ci, copies in (
                        (1, ((kT01, 0, 128, nc.scalar),)),
                        (2, None),
                        (-1, ()),
                        (0, ((qT01, 0, 128, nc.scalar),))):
                    if ci == -1:
                        for g in range(SEGC // 4):
                            acc = vps.tile([128, 4, 4, 64], F32, tag="vps")
                            for t4 in range(4):
                                t = g * 4 + t4
                                for c in range(DC):
                                    nc.tensor.matmul(
                                        acc[:, t4, 0:3, :],
                                        xT[:, c, t * 128:(t + 1) * 128],
                                        wqkv[:, c, 384:576],
                                        start=(c == 0), stop=(c == DC - 1))
                            tg = t0 + g * 4
                            nc.scalar.copy(V[:, tg:tg + 4, 0:3:2, 0:64],
                                           acc[:, :, 0:3:2, :])
                            nc.scalar.copy(V[:, tg:tg + 4, 1, 64:128],
                                           acc[:, :, 1, :])
                        continue
                    c0 = 128 * ci
                    acc = qkps.tile([128, SEG], F32, tag="qkps")
                    # c outer: the stationary wqkv chunk is loaded once per c
                    # and reused for both nb half-banks
                    for c in range(DC):
                        for nb in range(SEG // 512):
                            nc.tensor.matmul(
                                acc[:, nb * 512:(nb + 1) * 512],
                                wqkv[:, c, c0:c0 + 128],
                                xT[:, c, nb * 512:(nb + 1) * 512],
                                start=(c == 0), stop=(c == DC - 1))
                    if copies is None:
                        # q2/k2 are DUPLICATED into both partition halves so
                        # the h2 pass can run self-paired row-tiled matmuls
                        # (two query halves concurrently on the K=64 head).
                        nc.vector.tensor_copy(qT2[0:64, col0:col0 + SEG],
                                              acc[0:64, :])
                        nc.sync.dma_start(qT2[64:128, col0:col0 + SEG],
                                          qT2[0:64, col0:col0 + SEG])
                        ktmp = ktp.tile([128, SEG], F16, tag="ktmp")
                        nc.scalar.copy(ktmp[64:128, :], acc[64:128, :])
                        nc.vector.tensor_copy(kT2z[64:128, col0:col0 + SEG],
                                              ktmp[64:128, :])
                        nc.sync.dma_start(kT2z[0:64, col0:col0 + SEG],
                                          ktmp[64:128, :])
                        continue
                    for dst, lo, hi, eng in copies:
                        if eng is nc.scalar:
                            nc.scalar.copy(dst[lo:hi, col0:col0 + SEG],
                                           acc[lo:hi, :])
                        else:
                            nc.vector.tensor_copy(dst[lo:hi, col0:col0 + SEG],
                                                  acc[lo:hi, :])

        # ========= phase B: flash attention + fused output projection ====
        with tc.tile_pool(name="sps", bufs=3, space=bass.MemorySpace.PSUM) as sps, \
             tc.tile_pool(name="ops", bufs=1, space=bass.MemorySpace.PSUM) as ops, \
             tc.tile_pool(name="pp", bufs=4) as pp, \
             tc.tile_pool(name="ysbp", bufs=3) as ysbp, \
             tc.tile_pool(name="osbp", bufs=2) as osbp, \
             tc.tile_pool(name="rp", bufs=2) as rp, \
             tc.tile_pool(name="rbp", bufs=2) as rbp:
            pending = []   # deferred output-projection token chunks

            def emit_y(t):
                ts = slice(t * 128, (t + 1) * 128)
                y = sps.tile([128, D], F32, tag="s", name="y")
                # A01 carries h0 (rows 0..63) and h1 (rows 64..127) so one
                # K=128 contraction sums both heads; A2z is K=65 (h2+bias)
                for hh, (az, w) in enumerate(((A01, wout01), (A2z, wout2z))):
                    for c0, c1 in ((0, 512), (512, 768)):
                        nc.tensor.matmul(y[:, c0:c1], az[:, ts],
                                         w[:, c0:c1],
                                         start=(hh == 0), stop=(hh == 1))
                ysb = ysbp.tile([128, D], F32, tag="ysb", name="ysb")
                nc.scalar.copy(ysb[:], y[:])
                nc.sync.dma_start(y_d.ap()[ts, :], ysb[:])

            def exp_drain(p, s, kc):
                if kc in DVE_KC:
                    nc.vector.tensor_scalar(
                        p[:].bitcast(I16), s[:], EXP_A, EXP_B, MULT, ADD)
                else:
                    nc.scalar.activation(p[:], s[:], EXP, scale=SCALE)

            def normalize_pair(o2, q0):
                # o2 layout: cols 0:512 = h0 (rows 0..63 o, row 64 den),
                # cols 512:1024 = h1 (row 0 den, rows 64..127 o). ScalarE
                # frees the bank; DVE gathers dens at base partition 0
                # (recip input must sit at base 0 — base-64 APs NaN on HW);
                # GpSimd broadcasts + multiplies into A01's row halves.
                osb = osbp.tile([128, 2 * QCP], F32, tag="osb", name="osb")
                nc.scalar.copy(osb[:], o2[:, :])
                den = rp.tile([1, 2 * QCP], F32, tag="den")
                nc.vector.tensor_copy(den[:, 0:QCP], osb[64:65, 0:QCP])
                nc.vector.tensor_copy(den[:, QCP:], osb[0:1, QCP:])
                rc = rp.tile([1, 2 * QCP], F32, tag="rc")
                nc.vector.reciprocal_approx_fast(rc[:], den[:])
                rcb = rbp.tile([128, 2 * QCP], F32, tag="rcb")
                nc.gpsimd.partition_broadcast(rcb[:], rc[:])
                nc.gpsimd.tensor_mul(A01[0:64, q0:q0 + QCP],
                                     osb[0:64, 0:QCP], rcb[0:64, 0:QCP])
                nc.gpsimd.tensor_mul(A01[64:128, q0:q0 + QCP],
                                     osb[64:128, QCP:], rcb[64:128, QCP:])

            def normalize_h2(o, q0):
                osb = osbp.tile([65, QC], F32, tag="osb2", name="osb2")
                nc.scalar.copy(osb[:], o[0:65, :])
                den = rp.tile([1, QC], F32, tag="den")
                nc.vector.tensor_copy(den[:], osb[64:65, :])
                rc = rp.tile([1, QC], F32, tag="rc")
                nc.vector.reciprocal_approx_fast(rc[:], den[:])
                rcb = rbp.tile([64, QC], F32, tag="rcb")
                nc.gpsimd.partition_broadcast(rcb[:], rc[:])
                nc.gpsimd.tensor_mul(A2z[0:64, q0:q0 + QC],
                                     osb[0:64, :], rcb[:])

            QCP = 512  # pair-pass query block (h0+h1 share a 1024-wide tile)

            def head_emits(n):
                # before the kc loop: these y matmuls sit ahead of the
                # exp-gated AV in the PE FIFO and fill the pass spin-up
                # stall (a mid-loop emit at kc==0 queues BEHIND the
                # blocked AV and cannot)
                for _ in range(n):
                    if pending:
                        emit_y(pending.pop(0))

            def pair_pass(qbb, half):
                q0 = qbb * QC + half * QCP
                head_emits(2)
                o2 = ops.tile([128, 2 * QCP], F32, tag="o")
                for kc in range(NKC):
                    kk = slice(kc * 128, (kc + 1) * 128)
                    s2 = sps.tile([128, 2 * QCP], F32, tag="s")
                    nc.tensor.matmul(s2[:, 0:QCP], kT01[0:64, kk],
                                     qT01[0:64, q0:q0 + QCP],
                                     start=True, stop=True,
                                     tile_position=(0, 0))
                    nc.tensor.matmul(s2[:, QCP:], kT01[64:128, kk],
                                     qT01[64:128, q0:q0 + QCP],
                                     start=True, stop=True,
                                     tile_position=(64, 0))
                    p = pp.tile([128, 2 * QCP], F16, tag="p")
                    exp_drain(p, s2, kc)
                    nc.tensor.matmul(o2[:, 0:QCP], V[:, kc, 0, :],
                                     p[:, 0:QCP],
                                     start=(kc == 0), stop=(kc == NKC - 1))
                    nc.tensor.matmul(o2[:, QCP:], V[:, kc, 1, :],
                                     p[:, QCP:],
                                     start=(kc == 0), stop=(kc == NKC - 1))
                    if pending and kc == 16:
                        emit_y(pending.pop(0))
                normalize_pair(o2, q0)

            def h2_pass(qbb):
                q0 = qbb * QC
                head_emits(2)
                o = ops.tile([128, QC], F32, tag="o")
                for kc in range(NKC):
                    kk = slice(kc * 128, (kc + 1) * 128)
                    s = sps.tile([128, QC], F32, tag="s")
                    # self-paired: both query halves of h2 run concurrently
                    # in the two 64-row array halves (k2/q2 duplicated there)
                    nc.tensor.matmul(s[:, 0:512], kT2z[0:64, kk],
                                     qT2[0:64, q0:q0 + 512],
                                     start=True, stop=True,
                                     tile_position=(0, 0))
                    nc.tensor.matmul(s[:, 512:1024], kT2z[64:128, kk],
                                     qT2[64:128, q0 + 512:q0 + 1024],
                                     start=True, stop=True,
                                     tile_position=(64, 0))
                    p = pp.tile([128, QC], F16, tag="p")
                    exp_drain(p, s, kc)
                    for j in (0, 512):
                        nc.tensor.matmul(o[:, j:j + 512], V[:, kc, 2, :],
                                         p[:, j:j + 512],
                                         start=(kc == 0), stop=(kc == NKC - 1))
                    if pending and kc == 16:
                        emit_y(pending.pop(0))
                normalize_h2(o, q0)

            for qbb in range(NQC):
                if qbb < NQC - 1:
                    pair_pass(qbb, 0)
                    pair_pass(qbb, 1)
                    h2_pass(qbb)
                    pending.extend(range(qbb * 8, qbb * 8 + 8))
                else:
                    # last block: h2 first so its A_z is ready early, letting
                    # the first half's output chunks trickle into the second
                    # pair pass instead of serializing at the very end
                    h2_pass(qbb)
                    pair_pass(qbb, 0)
                    pending.extend(range(qbb * 8, qbb * 8 + 4))
                    pair_pass(qbb, 1)
                    pending.extend(range(qbb * 8 + 4, qbb * 8 + 8))
            for t in pending:
                emit_y(t)


def _get_nc():
    global _nc_cache
    if _nc_cache is None:
        _nc_cache = _build_module()
    return _nc_cache


def kernel(x, W_qkv, W_out, b_out):
    global LAST_RESULT
    x = np.asarray(x, dtype=np.float32)
    W_qkv = np.asarray(W_qkv, dtype=np.float32)
    W_out = np.asarray(W_out, dtype=np.float32)
    b_out = np.asarray(b_out, dtype=np.float32)

    in_maps = []
    for c in range(N_CORES):
        b, j = divmod(c, 4)
        h0 = 3 * j
        q0, k0, v0 = 64 * h0, D + 64 * h0, 2 * D + 64 * h0
        q01 = W_qkv[:, q0:q0 + 128]
        k01 = W_qkv[:, k0:k0 + 128]
        q2 = W_qkv[:, q0 + 128:q0 + 192]
        k2 = W_qkv[:, k0 + 128:k0 + 192]
        v012 = W_qkv[:, v0:v0 + 192]
        wqkv_slice = np.ascontiguousarray(np.concatenate(
            [q01, k01, q2, k2, v012], axis=1).astype(np.float16))
        r0 = 64 * h0
        bias_row = b_out[None, :] if j == 0 else np.zeros((1, D), np.float32)
        wout_slice = np.ascontiguousarray(np.concatenate(
            [W_out[r0:r0 + 192], bias_row], axis=0).astype(np.float16))
        in_maps.append({
            "x": np.ascontiguousarray(x[b].T.astype(np.float16)),
            "wqkv": wqkv_slice,
            "wout": wout_slice,
        })

    nc = _get_nc()
    kwargs = {}
    if TRACE:
        from concourse import bass_utils as _bu
        _bu.upload_artifacts = lambda tmpdir: "local://" + tmpdir
        kwargs["trace"] = True
        if TRACE_ALL_CORES:
            kwargs["trace_cores"] = list(range(N_CORES))
    res = run_bass_kernel_spmd(nc, in_maps, core_ids=list(range(N_CORES)), **kwargs)
    LAST_RESULT = res

    out = np.empty((B, N, D), dtype=np.float32)
    for b in range(B):
        out[b] = (res.results[4 * b + 0]["y"] + res.results[4 * b + 1]["y"]
                  + res.results[4 * b + 2]["y"] + res.results[4 * b + 3]["y"])
    return out


# revision 34
# speedup vs baseline: 1.1997x; 1.0963x over previous
"""Trainium2 Bass kernel v5 — paired K=64 attention + ScalarE/DVE exp split.

Multi-head attention (B=2, N=4096, D=768, H=12, d_head=64) on 8 NeuronCores.
Data-parallel over batch (4 cores per element), tensor-parallel over heads
(3 heads per core). Host sums the 4 partial outputs per batch element.

Key design points (each bench-validated on HW):
  * Softmax drain split: every [128, 1024] score tile leaves PSUM through
    exactly one instruction — ScalarE true exp (even key-chunks, 1113ns)
    or DVE Schraudolph fast-exp (odd chunks, one tensor_scalar:
    p = bitcast_fp16(int16(s * SCALE*1024/ln2 + 15360-61.5)), 1223ns).
    The -61.5 zeroes the MEAN multiplicative bias of the +-4% sawtooth —
    a systematic bias between DVE and ACT chunks would NOT cancel in the
    softmax normalization. Final rel-err 9.3e-3 vs the 2e-2 gate.
  * Score matmuls pair two K=64 heads in the 128-row PE array via
    tile_position (0,0)/(64,0) row tiling (concurrent, ~4ns apart): the
    h0/h1 passes pair the heads; the h2 pass pairs two query halves
    against duplicated k2/q2 partition halves.
  * sps bufs=3 sets the s-tile ring depth that hides the ~1.7us
    sem+exp latency chain; ops bufs=1 works because ScalarE frees o
    (rows 0..64 -> SBUF) immediately after the AV accumulation stops.
  * Inputs are cast to fp16 on the host and DMA straight into fp16 SBUF
    tiles (no on-device cast stream); k2 reaches its kT2z partition rows
    via an SBUF->SBUF DMA (engines cannot cross partitions, DMA can).
  * Normalize chain: ScalarE o-copy, DVE reciprocal (input MUST sit at
    base partition 0 — feeding a base-64 AP produced NaNs on HW),
    GpSimd broadcast + multiply into A_z.
  * Output projection token-chunks trickle into the PE stream mid-pass
    (kc 10/20/30) to fill exp-chain stalls.

Layouts (fp16): qT01/kT01 [128, N]: h0 rows 0..63, h1 rows 64..127.
qT2/kT2z [128, N]: q2/k2 duplicated in both 64-row halves. V
[128, NKC, 3, 128]: per (kc, h): cols 0..63 = V, col 64 = ones (softmax
denominator via the AV matmul), rest zeros. A_z[h] [128, N]: normalized
attention rows 0..63, h2 row 64 = ones (bias row). wout_z[h] [128, D]:
W_out rows for head h in 0..63 (+bias row 64 for h2).
wqkv host layout [768, 576]: [q01(128) | k01(128) | q2k2(128) | v(192)].
"""

import numpy as np

import concourse.bass as bass
import concourse.tile as tile
from concourse import mybir, bacc
from concourse.bass_utils import run_bass_kernel_spmd

F32 = mybir.dt.float32
F16 = mybir.dt.float16
I16 = mybir.dt.int16
EXP = mybir.ActivationFunctionType.Exp
MULT = mybir.AluOpType.mult
ADD = mybir.AluOpType.add

N_CORES = 8
B = 2
N = 4096
D = 768
H = 12
HD = 64  # head dim
SCALE = HD ** -0.5
DC = D // 128       # 6 contraction chunks
QC = 1024           # query block
NQC = N // QC       # 4
NKC = N // 128      # 32 key chunks

# Schraudolph fp16 exp: p = bitcast_fp16(int16(x*EA + EB)); EA folds the
# softmax 1/sqrt(d) scale. EB zeroes the MEAN multiplicative bias (a
# systematic bias between DVE- and ACT-computed key chunks would not
# cancel in the softmax normalization).
EXP_A = SCALE * 1024.0 / np.log(2.0)
EXP_B = 15 * 1024.0 - 61.5
# Which key-chunks the DVE takes (ScalarE true-exp handles the rest).
# Even split: ScalarE's faster per-tile drain (1336ns vs 1469ns) offsets
# the o->SBUF normalize copies and y drains it also carries.
DVE_KC = set(range(1, NKC, 2))

TRACE = False
TRACE_ALL_CORES = False
LAST_RESULT = None

_nc_cache = None


def _build_module():
    nc = bacc.Bacc("TRN2", target_bir_lowering=False, debug=False,
                   num_devices=N_CORES)
    x_d = nc.dram_tensor("x", [D, N], F16, kind="ExternalInput")
    wqkv_d = nc.dram_tensor("wqkv", [D, 576], F16, kind="ExternalInput")
    wout_d = nc.dram_tensor("wout", [193, D], F16, kind="ExternalInput")
    y_d = nc.dram_tensor("y", [N, D], F32, kind="ExternalOutput")

    with tile.TileContext(nc) as tc:
        _emit(nc, tc, x_d, wqkv_d, wout_d, y_d)
    nc.compile()
    return nc


def _emit(nc, tc, x_d, wqkv_d, wout_d, y_d):
    from contextlib import ExitStack
    ctx = ExitStack()
    with ctx:
        weights = ctx.enter_context(tc.tile_pool(name="weights", bufs=1))
        qkvp = ctx.enter_context(tc.tile_pool(name="qkv", bufs=1))
        apool = ctx.enter_context(tc.tile_pool(name="attnout", bufs=1))

        # --- weights / persistent activations ---------------------------
        wqkv = weights.tile([128, DC, 576], F16, tag="wqkv")
        wout01 = weights.tile([128, D], F16, tag="wout01")
        wout2z = weights.tile([65, D], F16, tag="wout2z")
        qT01 = qkvp.tile([128, N], F16, tag="qT01")
        qT2 = qkvp.tile([128, N], F16, tag="qT2")
        kT01 = qkvp.tile([128, N], F16, tag="kT01")
        kT2z = qkvp.tile([128, N], F16, tag="kT2z")
        V = qkvp.tile([128, NKC, 3, 128], F16, tag="V")
        A01 = apool.tile([128, N], F16, tag="A01")
        A2z = apool.tile([65, N], F16, tag="A2z")

        # --- weight DMAs (straight into fp16 tiles) ----------------------
        nc.sync.dma_start(
            wqkv[:], wqkv_d.ap().rearrange("(c p) m -> p c m", p=128))
        nc.sync.dma_start(wout01[:], wout_d.ap()[0:128, :])
        nc.sync.dma_start(wout2z[:], wout_d.ap()[128:193, :])

        # zero-fill padded halves (one-time, GpSimd) + ones columns/rows
        nc.gpsimd.memset(V[:], 0.0)
        nc.vector.memset(V[:, :, 0, 64:65], 1.0)
        nc.vector.memset(V[:, :, 1, 0:1], 1.0)
        nc.vector.memset(V[:, :, 2, 64:65], 1.0)
        nc.vector.memset(A2z[64:65, :], 1.0)

        # ================= phase A: fused qkv projections ================
        with tc.tile_pool(name="xT", bufs=2) as xTp, \
             tc.tile_pool(name="ktmp", bufs=2) as ktp, \
             tc.tile_pool(name="vps", bufs=2, space=bass.MemorySpace.PSUM) as vps, \
             tc.tile_pool(name="qkps", bufs=2, space=bass.MemorySpace.PSUM) as qkps:
            NSEG = 4
            SEG = N // NSEG
            SEGC = SEG // 128
            for seg in range(NSEG):
                t0 = seg * SEGC
                col0 = seg * SEG
                xT = xTp.tile([128, DC, SEG], F16, tag="xT")
                nc.sync.dma_start(
                    xT[:],
                    x_d.ap().rearrange("(c p) n -> p c n", p=128)
                    [:, :, col0:col0 + SEG])
                # fused projections; (k and v first so attention can start
                # before q finishes). copies: (dst, rows, engine).
                # ci=2 is the shared q2|k2 block: q2 sits in psum rows 0..63
                # (straight copy); k2 lands in rows 64..127 and is shifted to
                # kT2z rows 0..63 by an SBUF->SBUF DMA (engines cannot move
                # data across partitions; DMA can).
                for ci, copies in (
                        (1, ((kT01, 0, 128, nc.scalar),)),
                        (2, None),
                        (-1, ()),
                        (0, ((qT01, 0, 128, nc.scalar),))):
                    if ci == -1:
                        for g in range(SEGC // 4):
                            acc = vps.tile([128, 4, 4, 64], F32, tag="vps")
                            for t4 in range(4):
                                t = g * 4 + t4
                                for c in range(DC):
                                    nc.tensor.matmul(
                                        acc[:, t4, 0:3, :],
                                        xT[:, c, t * 128:(t + 1) * 128],
                                        wqkv[:, c, 384:576],
                                        start=(c == 0), stop=(c == DC - 1))
                            tg = t0 + g * 4
                            nc.scalar.copy(V[:, tg:tg + 4, 0:3:2, 0:64],
                                           acc[:, :, 0:3:2, :])
                            nc.scalar.copy(V[:, tg:tg + 4, 1, 64:128],
                                           acc[:, :, 1, :])
                        continue
                    c0 = 128 * ci
                    acc = qkps.tile([128, SEG], F32, tag="qkps")
                    # c outer: the stationary wqkv chunk is loaded once per c
                    # and reused for both nb half-banks
                    for c in range(DC):
                        for nb in range(SEG // 512):
                            nc.tensor.matmul(
                                acc[:, nb * 512:(nb + 1) * 512],
                                wqkv[:, c, c0:c0 + 128],
                                xT[:, c, nb * 512:(nb + 1) * 512],
                                start=(c == 0), stop=(c == DC - 1))
                    if copies is None:
                        # q2/k2 are DUPLICATED into both partition halves so
                        # the h2 pass can run self-paired row-tiled matmuls
                        # (two query halves concurrently on the K=64 head).
                        nc.vector.tensor_copy(qT2[0:64, col0:col0 + SEG],
                                              acc[0:64, :])
                        nc.sync.dma_start(qT2[64:128, col0:col0 + SEG],
                                          qT2[0:64, col0:col0 + SEG])
                        ktmp = ktp.tile([128, SEG], F16, tag="ktmp")
                        nc.scalar.copy(ktmp[64:128, :], acc[64:128, :])
                        nc.vector.tensor_copy(kT2z[64:128, col0:col0 + SEG],
                                              ktmp[64:128, :])
                        nc.sync.dma_start(kT2z[0:64, col0:col0 + SEG],
                                          ktmp[64:128, :])
                        continue
                    for dst, lo, hi, eng in copies:
                        if eng is nc.scalar:
                            nc.scalar.copy(dst[lo:hi, col0:col0 + SEG],
                                           acc[lo:hi, :])
                        else:
                            nc.vector.tensor_copy(dst[lo:hi, col0:col0 + SEG],
                                                  acc[lo:hi, :])

        # ========= phase B: flash attention + fused output projection ====
        with tc.tile_pool(name="sps", bufs=3, space=bass.MemorySpace.PSUM) as sps, \
             tc.tile_pool(name="ops", bufs=1, space=bass.MemorySpace.PSUM) as ops, \
             tc.tile_pool(name="pp", bufs=4) as pp, \
             tc.tile_pool(name="ysbp", bufs=3) as ysbp, \
             tc.tile_pool(name="osbp", bufs=2) as osbp, \
             tc.tile_pool(name="rp", bufs=2) as rp, \
             tc.tile_pool(name="rbp", bufs=2) as rbp:
            pending = []   # deferred output-projection token chunks

            def emit_y(t):
                ts = slice(t * 128, (t + 1) * 128)
                y = sps.tile([128, D], F32, tag="s", name="y")
                # A01 carries h0 (rows 0..63) and h1 (rows 64..127) so one
                # K=128 contraction sums both heads; A2z is K=65 (h2+bias)
                for hh, (az, w) in enumerate(((A01, wout01), (A2z, wout2z))):
                    for c0, c1 in ((0, 512), (512, 768)):
                        nc.tensor.matmul(y[:, c0:c1], az[:, ts],
                                         w[:, c0:c1],
                                         start=(hh == 0), stop=(hh == 1))
                ysb = ysbp.tile([128, D], F32, tag="ysb", name="ysb")
                nc.scalar.copy(ysb[:, 0:384], y[:, 0:384])
                nc.vector.tensor_copy(ysb[:, 384:768], y[:, 384:768])
                nc.sync.dma_start(y_d.ap()[ts, :], ysb[:])

            def exp_drain(p, s, kc):
                if kc in DVE_KC:
                    nc.vector.tensor_scalar(
                        p[:].bitcast(I16), s[:], EXP_A, EXP_B, MULT, ADD)
                else:
                    nc.scalar.activation(p[:], s[:], EXP, scale=SCALE)

            def normalize_pair(o2, q0, last=False):
                # o2 layout: cols 0:512 = h0 (rows 0..63 o, row 64 den),
                # cols 512:1024 = h1 (row 0 den, rows 64..127 o). ScalarE
                # frees the bank; DVE gathers dens at base partition 0
                # (recip input must sit at base 0 — base-64 APs NaN on HW);
                # GpSimd broadcasts + multiplies into A01's row halves.
                osb = osbp.tile([128, 2 * QCP], F32, tag="osb", name="osb")
                nc.scalar.copy(osb[:], o2[:, :])
                den = rp.tile([1, 2 * QCP], F32, tag="den")
                nc.vector.tensor_copy(den[:, 0:QCP], osb[64:65, 0:QCP])
                nc.vector.tensor_copy(den[:, QCP:], osb[0:1, QCP:])
                rc = rp.tile([1, 2 * QCP], F32, tag="rc")
                nc.vector.reciprocal_approx_fast(rc[:], den[:])
                rcb = rbp.tile([128, 2 * QCP], F32, tag="rcb")
                nc.gpsimd.partition_broadcast(rcb[:], rc[:])
                eng = nc.vector if last else nc.gpsimd
                eng.tensor_mul(A01[0:64, q0:q0 + QCP],
                               osb[0:64, 0:QCP], rcb[0:64, 0:QCP])
                eng.tensor_mul(A01[64:128, q0:q0 + QCP],
                               osb[64:128, QCP:], rcb[64:128, QCP:])

            def normalize_h2(o, q0):
                osb = osbp.tile([65, QC], F32, tag="osb2", name="osb2")
                nc.scalar.copy(osb[:], o[0:65, :])
                den = rp.tile([1, QC], F32, tag="den")
                nc.vector.tensor_copy(den[:], osb[64:65, :])
                rc = rp.tile([1, QC], F32, tag="rc")
                nc.vector.reciprocal_approx_fast(rc[:], den[:])
                rcb = rbp.tile([64, QC], F32, tag="rcb")
                nc.gpsimd.partition_broadcast(rcb[:], rc[:])
                nc.gpsimd.tensor_mul(A2z[0:64, q0:q0 + QC],
                                     osb[0:64, :], rcb[:])

            QCP = 512  # pair-pass query block (h0+h1 share a 1024-wide tile)

            def pair_pass(qbb, half, last=False):
                q0 = qbb * QC + half * QCP
                o2 = ops.tile([128, 2 * QCP], F32, tag="o")
                for kc in range(NKC):
                    kk = slice(kc * 128, (kc + 1) * 128)
                    s2 = sps.tile([128, 2 * QCP], F32, tag="s")
                    nc.tensor.matmul(s2[:, 0:QCP], kT01[0:64, kk],
                                     qT01[0:64, q0:q0 + QCP],
                                     start=True, stop=True,
                                     tile_position=(0, 0))
                    nc.tensor.matmul(s2[:, QCP:], kT01[64:128, kk],
                                     qT01[64:128, q0:q0 + QCP],
                                     start=True, stop=True,
                                     tile_position=(64, 0))
                    p = pp.tile([128, 2 * QCP], F16, tag="p")
                    exp_drain(p, s2, kc)
                    nc.tensor.matmul(o2[:, 0:QCP], V[:, kc, 0, :],
                                     p[:, 0:QCP],
                                     start=(kc == 0), stop=(kc == NKC - 1))
                    nc.tensor.matmul(o2[:, QCP:], V[:, kc, 1, :],
                                     p[:, QCP:],
                                     start=(kc == 0), stop=(kc == NKC - 1))
                    if pending and kc in (10, 20, 30):
                        emit_y(pending.pop(0))
                normalize_pair(o2, q0, last)

            def h2_pass(qbb):
                q0 = qbb * QC
                o = ops.tile([128, QC], F32, tag="o")
                for kc in range(NKC):
                    kk = slice(kc * 128, (kc + 1) * 128)
                    s = sps.tile([128, QC], F32, tag="s")
                    # self-paired: both query halves of h2 run concurrently
                    # in the two 64-row array halves (k2/q2 duplicated there)
                    nc.tensor.matmul(s[:, 0:512], kT2z[0:64, kk],
                                     qT2[0:64, q0:q0 + 512],
                                     start=True, stop=True,
                                     tile_position=(0, 0))
                    nc.tensor.matmul(s[:, 512:1024], kT2z[64:128, kk],
                                     qT2[64:128, q0 + 512:q0 + 1024],
                                     start=True, stop=True,
                                     tile_position=(64, 0))
                    p = pp.tile([128, QC], F16, tag="p")
                    exp_drain(p, s, kc)
                    for j in (0, 512):
                        nc.tensor.matmul(o[:, j:j + 512], V[:, kc, 2, :],
                                         p[:, j:j + 512],
                                         start=(kc == 0), stop=(kc == NKC - 1))
                    if pending and kc in (10, 20, 30):
                        emit_y(pending.pop(0))
                normalize_h2(o, q0)

            for qbb in range(NQC):
                if qbb < NQC - 1:
                    pair_pass(qbb, 0)
                    pair_pass(qbb, 1)
                    h2_pass(qbb)
                    pending.extend(range(qbb * 8, qbb * 8 + 8))
                else:
                    # last block: h2 first so its A_z is ready early, letting
                    # the first half's output chunks trickle into the second
                    # pair pass instead of serializing at the very end
                    h2_pass(qbb)
                    pair_pass(qbb, 0)
                    pending.extend(range(qbb * 8, qbb * 8 + 4))
                    pair_pass(qbb, 1, last=True)
                    pending.extend(range(qbb * 8 + 4, qbb * 8 + 8))
            for t in pending:
                emit_y(t)


def _get_nc():
    global _nc_cache
    if _nc_cache is None:
        _nc_cache = _build_module()
    return _nc_cache


def kernel(x, W_qkv, W_out, b_out):
    global LAST_RESULT
    x = np.asarray(x, dtype=np.float32)
    W_qkv = np.asarray(W_qkv, dtype=np.float32)
    W_out = np.asarray(W_out, dtype=np.float32)
    b_out = np.asarray(b_out, dtype=np.float32)

    in_maps = []
    for c in range(N_CORES):
        b, j = divmod(c, 4)
        h0 = 3 * j
        q0, k0, v0 = 64 * h0, D + 64 * h0, 2 * D + 64 * h0
        q01 = W_qkv[:, q0:q0 + 128]
        k01 = W_qkv[:, k0:k0 + 128]
        q2 = W_qkv[:, q0 + 128:q0 + 192]
        k2 = W_qkv[:, k0 + 128:k0 + 192]
        v012 = W_qkv[:, v0:v0 + 192]
        wqkv_slice = np.ascontiguousarray(np.concatenate(
            [q01, k01, q2, k2, v012], axis=1).astype(np.float16))
        r0 = 64 * h0
        bias_row = b_out[None, :] if j == 0 else np.zeros((1, D), np.float32)
        wout_slice = np.ascontiguousarray(np.concatenate(
            [W_out[r0:r0 + 192], bias_row], axis=0).astype(np.float16))
        in_maps.append({
            "x": np.ascontiguousarray(x[b].T.astype(np.float16)),
            "wqkv": wqkv_slice,
            "wout": wout_slice,
        })

    nc = _get_nc()
    kwargs = {}
    if TRACE:
        from concourse import bass_utils as _bu
        _bu.upload_artifacts = lambda tmpdir: "local://" + tmpdir
        kwargs["trace"] = True
        if TRACE_ALL_CORES:
            kwargs["trace_cores"] = list(range(N_CORES))
    res = run_bass_kernel_spmd(nc, in_maps, core_ids=list(range(N_CORES)), **kwargs)
    LAST_RESULT = res

    out = np.empty((B, N, D), dtype=np.float32)
    for b in range(B):
        out[b] = (res.results[4 * b + 0]["y"] + res.results[4 * b + 1]["y"]
                  + res.results[4 * b + 2]["y"] + res.results[4 * b + 3]["y"])
    return out
